# revision 24
# baseline (speedup 1.0000x reference)
"""MoE (top-2 of 8 experts + shared SwiGLU) Trainium2 kernel, expert-parallel.

Strategy (8 NeuronCores):
  - Host computes the gate in true fp32 (matches the reference's fp32
    softmax/top-2 ordering; min top2/3 prob gap for this input is 1.5e-6,
    ~40x above fp32 matmul noise) and sorts tokens by expert.
  - Expert-parallel: core e owns routed expert e. The host ships, per core,
    the expert's weights plus the dispatched token matrix ALREADY gathered
    and transposed (xrT = x[ids_e].T in fp16, padded to CAP columns), so the
    device does zero gathers/transposes - it is a pure GEMM pipeline.
  - Shared SwiGLU is data-parallel: core c also processes tokens
    [c*1024, (c+1)*1024) through the shared experts (no communication).
  - Each core writes two compact outputs: z [1024, D] (shared) and
    y [CAP, D] (unweighted routed expert output). The host applies the
    routing weights and scatters:  out[ids_e] += w_e[:,None] * y[:n_e].
  - CAP = 2048 so the routed phases are 4 clean 512-wide chunks; the few
    tokens past 2048 on over-popular experts (~100 rows total) are computed
    on the host in fp32 and added during the combine.

Phase order S1 -> R1 -> S2 -> R2: each phase's inputs are finished at
least one phase earlier, so the PE never stalls at a boundary. All pools
live for the whole program (R1 iterates chunk-outer so xr streams through
a small rotating pool, and R1 re-streams its mm1 weights once per chunk -
DMA has bandwidth to spare, SBUF does not), so there are no pool alloc/
release barriers on the instruction queues; weight prefetch flows through
plain buffer rotation. All matmuls are fp16 with fp32 PSUM accumulation
at N=512 moving chunks; the PE streams at peak rate end to end.
"""

import math
from contextlib import ExitStack
from functools import lru_cache

import numpy as np

import concourse.bass as bass
import concourse.mybir as mybir
import concourse.tile as tile
from concourse import bacc
from concourse.bass_utils import run_bass_kernel_spmd

F32 = mybir.dt.float32
F16 = mybir.dt.float16
AF = mybir.ActivationFunctionType
OP = mybir.AluOpType

P = 128
N_CORES = 8

# Problem dims (B=4, S=2048, D=2048, E=8, I=1408, SI=2816)
T = 8192
D = 2048
E = 8
I = 1408
SI = 2816
TSC = T // N_CORES          # shared-slice tokens per core
CAP = 2048                  # routed token capacity per core (4 chunks of 512)

ND = D // P                 # 16
NI = I // P                 # 11
NSI = SI // P               # 22
NCT = CAP // P              # 16
DCH = 512                   # moving chunk over d (mm2 outputs)
TCH = 512                   # moving chunk over tokens (mm1)
IGRP = 2                    # i-tiles per batched mm1 weight DMA


def mm1_unit(nc, pools, w1b, w3b, q, rhs, ghalf, i, c0):
    """One (i-tile, token-chunk) SwiGLU unit: 32 matmuls + silu + mult."""
    ps, sb = pools["ps1"], pools["sg"]
    h1 = ps.tile([P, TCH], F32, space="PSUM", name="h1", tag="h1")
    h3 = ps.tile([P, TCH], F32, space="PSUM", name="h3", tag="h3")
    for d in range(ND):
        nc.tensor.matmul(out=h1[:], lhsT=w1b[:, d, q * P:(q + 1) * P],
                         rhs=rhs[:, d, :], start=(d == 0), stop=(d == ND - 1))
    for d in range(ND):
        nc.tensor.matmul(out=h3[:], lhsT=w3b[:, d, q * P:(q + 1) * P],
                         rhs=rhs[:, d, :], start=(d == 0), stop=(d == ND - 1))
    sg = sb.tile([P, TCH], F32, name="sg", tag="sg")
    nc.scalar.activation(sg[:], h1[:], AF.Silu)
    nc.vector.tensor_tensor(out=ghalf[:, i, c0:c0 + TCH],
                            in0=sg[:], in1=h3[:], op=OP.mult)


def m1w_group(nc, pools, wA, wB, g, n_half, split_first):
    """DMA one batched (w1, w3) stationary-weight group into the m1w pool."""
    i0 = g * IGRP
    ng = min(IGRP, n_half - i0)
    wp = pools["m1w"]
    w1b = wp.tile([P, ND, IGRP * P], F16, name="w1b", tag="w1b")
    w3b = wp.tile([P, ND, IGRP * P], F16, name="w3b", tag="w3b")
    wAr = wA[:].rearrange("dt p i -> p dt i")
    wBr = wB[:].rearrange("dt p i -> p dt i")
    nq = ng if (split_first and g == 0) else 1
    for s in range(nq):
        lo, hi = s * P, ng * P if s == nq - 1 else (s + 1) * P
        nc.sync.dma_start(out=w1b[:, :, lo:hi],
                          in_=wAr[:, :, i0 * P + lo:i0 * P + hi])
        nc.sync.dma_start(out=w3b[:, :, lo:hi],
                          in_=wBr[:, :, i0 * P + lo:i0 * P + hi])
    return w1b, w3b, ng


def mm2(nc, pools, g_sb, w2L, out, n_half, n_tok):
    """out[t, d] = sum_i g[i, t] * w2[d, i], written in [P, DCH] tiles."""
    wp, osb, ps = pools["m2w"], pools["osb"], pools["ps2"]
    for ch in range(D // DCH):
        w2t = wp.tile([P, NSI, DCH], F16, name="w2t", tag="w2t")
        w2r = w2L[:].rearrange("i p d -> p i d")
        half = (n_half + 3) // 4
        for h0 in range(0, n_half, half):
            h1 = min(h0 + half, n_half)
            nc.sync.dma_start(
                out=w2t[:, h0:h1, :],
                in_=w2r[:, h0:h1, ch * DCH:(ch + 1) * DCH])
        for tj in range(n_tok // P):
            op = ps.tile([P, DCH], F32, space="PSUM", name="op", tag="op")
            for i in range(n_half):
                nc.tensor.matmul(
                    out=op[:], lhsT=g_sb[:, i, tj * P:(tj + 1) * P],
                    rhs=w2t[:, i, :],
                    start=(i == 0), stop=(i == n_half - 1))
            o_sb = osb.tile([P, DCH], F32, name="osb", tag="osb")
            nc.scalar.copy(o_sb[:], op[:])
            nc.sync.dma_start(
                out=out[tj * P:(tj + 1) * P, ch * DCH:(ch + 1) * DCH],
                in_=o_sb[:])


def build_moe(nc, tc, ctx, io):
    xsT, xrT = io["xsT"], io["xrT"]
    w1L, w3L, w2L = io["w1L"], io["w3L"], io["w2L"]
    sw1L, sw3L, sw2L = io["sw1L"], io["sw3L"], io["sw2L"]
    z_out, y_out = io["z"], io["y"]

    # SBUF per partition: gs 44 + xrc 32 + m1w 32 + sg 4 + m2w 45 + osb 4
    # = 161 KB resident, plus xs 32 (S1) swapped for ge 44 (R1..R2).
    gs_pool = tc.alloc_tile_pool(name="gs", bufs=1, side="left")
    xrc_pool = tc.alloc_tile_pool(name="xrc", bufs=2, side="left")
    m1w_pool = tc.alloc_tile_pool(name="m1w", bufs=2, side="left")
    sg_pool = tc.alloc_tile_pool(name="sg", bufs=2, side="left")
    m2w_pool = tc.alloc_tile_pool(name="m2w", bufs=2, side="right")
    osb_pool = tc.alloc_tile_pool(name="osb", bufs=2, side="right")
    xs_pool = tc.alloc_tile_pool(name="xs", bufs=1, side="left")
    ps1_pool = tc.alloc_tile_pool(name="ps1", bufs=2, space="PSUM")
    ps2_pool = tc.alloc_tile_pool(name="ps2", bufs=2, space="PSUM")
    pools = dict(m1w=m1w_pool, sg=sg_pool, m2w=m2w_pool, osb=osb_pool,
                 ps1=ps1_pool, ps2=ps2_pool)

    gs = gs_pool.tile([P, NSI, TSC], F16)
    xs_sb = xs_pool.tile([P, ND, TSC], F16)

    # ---- Phase S1: gs = silu(sw1 x)*(sw3 x), xs resident, chunk-inner ----
    xsr = xsT[:].rearrange("(dt p) c -> p dt c", p=P)
    nc.sync.dma_start(out=xs_sb[:, :8, 0:TCH], in_=xsr[:, :8, 0:TCH])
    nc.sync.dma_start(out=xs_sb[:, 8:, 0:TCH], in_=xsr[:, 8:, 0:TCH])
    # warm the PE (HAM un-throttles after ~3.4us of activity) with junk
    # matmuls on a memset tile while the first real DMAs land
    warm = sg_pool.tile([P, TCH], F32, name="warm", tag="sg")
    nc.vector.memset(warm[:, :P], 0.0)
    wps = pools["ps2"].tile([P, DCH], F32, space="PSUM", name="wps", tag="op")
    wsrc = warm[:, :64].bitcast(F16)
    for _ in range(100):
        nc.tensor.matmul(out=wps[:, :P], lhsT=wsrc, rhs=wsrc,
                         start=True, stop=True)
    for g in range(math.ceil(NSI / IGRP)):
        w1b, w3b, ng = m1w_group(nc, pools, sw1L, sw3L, g, NSI, True)
        if g == 0:
            nc.sync.dma_start(out=xs_sb[:, :, TCH:TSC], in_=xsr[:, :, TCH:TSC])
            # chunk-major for the first group: both si-tiles work on chunk 0
            # while the chunk-1 DMA is still in flight
            units = [(q, c0) for c0 in range(0, TSC, TCH) for q in range(ng)]
        else:
            units = [(q, c0) for q in range(ng) for c0 in range(0, TSC, TCH)]
        for q, c0 in units:
            mm1_unit(nc, pools, w1b, w3b, q,
                     xs_sb[:, :, c0:c0 + TCH], gs, g * IGRP + q, c0)
    xs_pool.release()

    # ---- Phase R1: ge = silu(w1 xr)*(w3 xr), chunk-outer, xr streamed ----
    ge_pool = tc.alloc_tile_pool(name="ge", bufs=1, side="left")
    ge = ge_pool.tile([P, NI, CAP], F16)
    xrr = xrT[:].rearrange("(dt p) c -> p dt c", p=P)
    for c0 in range(0, CAP, TCH):
        xr_c = xrc_pool.tile([P, ND, TCH], F16, name="xrc", tag="xrc")
        nc.sync.dma_start(out=xr_c[:], in_=xrr[:, :, c0:c0 + TCH])
        for g in range(math.ceil(NI / IGRP)):
            w1b, w3b, ng = m1w_group(nc, pools, w1L, w3L, g, NI, False)
            for q in range(ng):
                mm1_unit(nc, pools, w1b, w3b, q, xr_c[:], ge, g * IGRP + q, c0)

    # ---- Phase S2: z = gs @ sw2 ----
    mm2(nc, pools, gs, sw2L, z_out, NSI, TSC)

    # ---- Phase R2: y = ge @ w2 ----
    mm2(nc, pools, ge, w2L, y_out, NI, CAP)

    ge_pool.release()
    sg_pool.release()
    m1w_pool.release()
    xrc_pool.release()
    gs_pool.release()
    osb_pool.release()
    m2w_pool.release()
    ps2_pool.release()
    ps1_pool.release()


def _declare_io(nc):
    io = {}
    io["xsT"] = nc.dram_tensor("xsT", [D, TSC], F16, kind="ExternalInput").ap()
    io["xrT"] = nc.dram_tensor("xrT", [D, CAP], F16, kind="ExternalInput").ap()
    io["w1L"] = nc.dram_tensor("w1L", [ND, P, I], F16, kind="ExternalInput").ap()
    io["w3L"] = nc.dram_tensor("w3L", [ND, P, I], F16, kind="ExternalInput").ap()
    io["w2L"] = nc.dram_tensor("w2L", [NI, P, D], F16, kind="ExternalInput").ap()
    io["sw1L"] = nc.dram_tensor("sw1L", [ND, P, SI], F16, kind="ExternalInput").ap()
    io["sw3L"] = nc.dram_tensor("sw3L", [ND, P, SI], F16, kind="ExternalInput").ap()
    io["sw2L"] = nc.dram_tensor("sw2L", [NSI, P, D], F16, kind="ExternalInput").ap()
    io["z"] = nc.dram_tensor("z", [TSC, D], F32, kind="ExternalOutput").ap()
    io["y"] = nc.dram_tensor("y", [CAP, D], F32, kind="ExternalOutput").ap()
    return io


@lru_cache(maxsize=1)
def _build():
    nc = bacc.Bacc("TRN2", target_bir_lowering=False, debug=False,
                   num_devices=N_CORES)
    io = _declare_io(nc)
    with tile.TileContext(nc) as tc:
        with ExitStack() as ctx:
            build_moe(nc, tc, ctx, io)
    nc.compile()
    return nc


def host_gate(xt, gate_w):
    """fp32 gate + top-2, matching jax.nn.softmax + lax.top_k semantics."""
    logits = (xt @ gate_w.T.astype(np.float32)).astype(np.float32)
    m = logits.max(axis=1, keepdims=True)
    ex = np.exp(logits - m, dtype=np.float32)
    p = ex / ex.sum(axis=1, keepdims=True, dtype=np.float32)
    # stable argsort of -p == top_k tie-breaking (lower index wins ties)
    order = np.argsort(-p, axis=1, kind="stable")[:, :2]
    return p.astype(np.float32), order


def make_in_maps(x, gate_w, w1, w2, w3, sw1, sw2, sw3):
    xt = np.ascontiguousarray(x.reshape(T, D)).astype(np.float32, copy=False)
    p, order = host_gate(xt, gate_w)

    xT16 = np.ascontiguousarray(xt.T).astype(np.float16)  # [D, T]
    f16 = lambda a: np.ascontiguousarray(a).astype(np.float16)
    shared = dict(
        sw1L=f16(sw1.T).reshape(ND, P, SI),
        sw3L=f16(sw3.T).reshape(ND, P, SI),
        sw2L=f16(sw2.T).reshape(NSI, P, D),
    )
    in_maps = []
    ids_all, w_all, ov_all = [], [], []
    for c in range(N_CORES):
        ids = np.nonzero((order == c).any(axis=1))[0]
        ids_all.append(ids[:CAP])
        w_all.append(p[ids[:CAP], c])
        ov_all.append((ids[CAP:], p[ids[CAP:], c]))
        xrT = np.zeros((D, CAP), np.float16)
        xrT[:, :min(len(ids), CAP)] = xT16[:, ids[:CAP]]
        in_maps.append(dict(
            xsT=np.ascontiguousarray(xT16[:, c * TSC:(c + 1) * TSC]),
            xrT=xrT,
            w1L=f16(w1[c].T).reshape(ND, P, I),
            w3L=f16(w3[c].T).reshape(ND, P, I),
            w2L=f16(w2[c].T).reshape(NI, P, D),
            **shared,
        ))
    return in_maps, ids_all, w_all, ov_all


def _silu(v):
    return v / (1.0 + np.exp(-v))


def combine(res, ids_all, w_all, ov_all, xt, w1, w2, w3, shape):
    out = np.concatenate(
        [res.results[c]["z"] for c in range(N_CORES)], axis=0)  # [T, D] fp32
    for c in range(N_CORES):
        ids, w = ids_all[c], w_all[c]
        out[ids] += w[:, None] * res.results[c]["y"][:len(ids)]
        ov_ids, ov_w = ov_all[c]
        if len(ov_ids):  # overflow rows beyond CAP: exact fp32 on host
            xo = xt[ov_ids]
            h = _silu(xo @ w1[c].T) * (xo @ w3[c].T)
            out[ov_ids] += ov_w[:, None] * (h @ w2[c].T)
    return out.reshape(shape)


def kernel(x, gate_w, w1, w2, w3, sw1, sw2, sw3):
    nc = _build()
    xt = np.ascontiguousarray(x.reshape(T, D)).astype(np.float32, copy=False)
    in_maps, ids_all, w_all, ov_all = make_in_maps(
        x, gate_w, w1, w2, w3, sw1, sw2, sw3)
    res = run_bass_kernel_spmd(nc, in_maps, core_ids=list(range(N_CORES)))
    return combine(res, ids_all, w_all, ov_all, xt,
                   np.asarray(w1, np.float32), np.asarray(w2, np.float32),
                   np.asarray(w3, np.float32), x.shape)


# revision 25
# speedup vs baseline: 1.1950x; 1.1950x over previous
"""MoE (top-2 of 8 experts + shared SwiGLU) Trainium2 kernel, expert-parallel.

Strategy (8 NeuronCores):
  - Host computes the gate in true fp32 (matches the reference's fp32
    softmax/top-2 ordering; min top2/3 prob gap for this input is 1.5e-6,
    ~40x above fp32 matmul noise) and sorts tokens by expert.
  - Expert-parallel: core e owns routed expert e. The host ships, per core,
    the expert's weights plus the dispatched token matrix ALREADY gathered
    and transposed (xrT = x[ids_e].T in fp16, padded to CAP columns), so the
    device does zero gathers/transposes - it is a pure GEMM pipeline.
  - Shared SwiGLU is data-parallel: core c also processes tokens
    [c*1024, (c+1)*1024) through the shared experts (no communication).
  - Each core writes two compact outputs: z [1024, D] (shared) and
    y [CAP, D] (unweighted routed expert output). The host applies the
    routing weights and scatters:  out[ids_e] += w_e[:,None] * y[:n_e].
  - CAP = 2048 so the routed phases are 4 clean 512-wide chunks; the few
    tokens past 2048 on over-popular experts (~100 rows total) are computed
    on the host in fp32 and added during the combine.

Phase order S1 -> R1 -> S2 -> R2: each phase's inputs are finished at
least one phase earlier, so the PE never stalls at a boundary. All pools
live for the whole program (R1 iterates chunk-outer so xr streams through
a small rotating pool, and R1 re-streams its mm1 weights once per chunk -
DMA has bandwidth to spare, SBUF does not), so there are no pool alloc/
release barriers on the instruction queues; weight prefetch flows through
plain buffer rotation. All matmuls are fp16 with fp32 PSUM accumulation
at N=512 moving chunks; the PE streams at peak rate end to end.
"""

import math
from contextlib import ExitStack
from functools import lru_cache

import numpy as np

import concourse.bass as bass
import concourse.mybir as mybir
import concourse.tile as tile
from concourse import bacc
from concourse.bass_utils import run_bass_kernel_spmd

F32 = mybir.dt.float32
F16 = mybir.dt.float16
AF = mybir.ActivationFunctionType
OP = mybir.AluOpType

P = 128
N_CORES = 8

# Problem dims (B=4, S=2048, D=2048, E=8, I=1408, SI=2816)
T = 8192
D = 2048
E = 8
I = 1408
SI = 2816
TSC = T // N_CORES          # shared-slice tokens per core
CAP = 2048                  # routed token capacity per core (4 chunks of 512)

ND = D // P                 # 16
NI = I // P                 # 11
NSI = SI // P               # 22
NCT = CAP // P              # 16
DCH = 512                   # moving chunk over d (mm2 outputs)
TCH = 512                   # moving chunk over tokens (mm1)
IGRP = 2                    # i-tiles per batched mm1 weight DMA


def mm1_unit(nc, pools, w1b, w3b, q, rhs, ghalf, i, c0):
    """One (i-tile, token-chunk) SwiGLU unit: 32 matmuls + silu + mult."""
    ps, sb = pools["ps1"], pools["sg"]
    h1 = ps.tile([P, TCH], F32, space="PSUM", name="h1", tag="h1")
    h3 = ps.tile([P, TCH], F32, space="PSUM", name="h3", tag="h3")
    for d in range(ND):
        nc.tensor.matmul(out=h1[:], lhsT=w1b[:, d, q * P:(q + 1) * P],
                         rhs=rhs[:, d, :], start=(d == 0), stop=(d == ND - 1))
    for d in range(ND):
        nc.tensor.matmul(out=h3[:], lhsT=w3b[:, d, q * P:(q + 1) * P],
                         rhs=rhs[:, d, :], start=(d == 0), stop=(d == ND - 1))
    sg = sb.tile([P, TCH], F32, name="sg", tag="sg")
    nc.scalar.activation(sg[:], h1[:], AF.Silu)
    nc.vector.tensor_tensor(out=ghalf[:, i, c0:c0 + TCH],
                            in0=sg[:], in1=h3[:], op=OP.mult)


def m1w_group(nc, pools, wA, wB, g, n_half, split_first):
    """DMA one batched (w1, w3) stationary-weight group into the m1w pool."""
    i0 = g * IGRP
    ng = min(IGRP, n_half - i0)
    wp = pools["m1w"]
    w1b = wp.tile([P, ND, IGRP * P], F16, name="w1b", tag="w1b")
    w3b = wp.tile([P, ND, IGRP * P], F16, name="w3b", tag="w3b")
    wAr = wA[:].rearrange("dt p i -> p dt i")
    wBr = wB[:].rearrange("dt p i -> p dt i")
    nq = ng if (split_first and g == 0) else 1
    for s in range(nq):
        lo, hi = s * P, ng * P if s == nq - 1 else (s + 1) * P
        nc.sync.dma_start(out=w1b[:, :, lo:hi],
                          in_=wAr[:, :, i0 * P + lo:i0 * P + hi])
        nc.sync.dma_start(out=w3b[:, :, lo:hi],
                          in_=wBr[:, :, i0 * P + lo:i0 * P + hi])
    return w1b, w3b, ng


def mm2(nc, pools, g_sb, w2L, out, n_half, n_tok):
    """out[t, d] = sum_i g[i, t] * w2[d, i], written in [P, DCH] tiles."""
    wp, osb, ps = pools["m2w"], pools["osb"], pools["ps2"]
    for ch in range(D // DCH):
        w2t = wp.tile([P, NSI, DCH], F16, name="w2t", tag="w2t")
        w2r = w2L[:].rearrange("i p d -> p i d")
        half = (n_half + 3) // 4
        for h0 in range(0, n_half, half):
            h1 = min(h0 + half, n_half)
            nc.sync.dma_start(
                out=w2t[:, h0:h1, :],
                in_=w2r[:, h0:h1, ch * DCH:(ch + 1) * DCH])
        for tj in range(n_tok // P):
            op = ps.tile([P, DCH], F32, space="PSUM", name="op", tag="op")
            for i in range(n_half):
                nc.tensor.matmul(
                    out=op[:], lhsT=g_sb[:, i, tj * P:(tj + 1) * P],
                    rhs=w2t[:, i, :],
                    start=(i == 0), stop=(i == n_half - 1))
            o_sb = osb.tile([P, DCH], F32, name="osb", tag="osb")
            nc.scalar.copy(o_sb[:], op[:])
            nc.sync.dma_start(
                out=out[tj * P:(tj + 1) * P, ch * DCH:(ch + 1) * DCH],
                in_=o_sb[:])


def build_moe(nc, tc, ctx, io):
    xsT, xrT = io["xsT"], io["xrT"]
    w1L, w3L, w2L = io["w1L"], io["w3L"], io["w2L"]
    sw1L, sw3L, sw2L = io["sw1L"], io["sw3L"], io["sw2L"]
    z_out, y_out = io["z"], io["y"]

    # SBUF per partition: gs 44 + xrc 32 + m1w 32 + sg 4 + m2w 45 + osb 4
    # = 161 KB resident, plus xs 32 (S1) swapped for ge 44 (R1..R2).
    gs_pool = tc.alloc_tile_pool(name="gs", bufs=1, side="left")
    xrc_pool = tc.alloc_tile_pool(name="xrc", bufs=2, side="left")
    m1w_pool = tc.alloc_tile_pool(name="m1w", bufs=2, side="left")
    sg_pool = tc.alloc_tile_pool(name="sg", bufs=2, side="left")
    m2w_pool = tc.alloc_tile_pool(name="m2w", bufs=2, side="right")
    osb_pool = tc.alloc_tile_pool(name="osb", bufs=2, side="right")
    xs_pool = tc.alloc_tile_pool(name="xs", bufs=1, side="left")
    ps1_pool = tc.alloc_tile_pool(name="ps1", bufs=2, space="PSUM")
    ps2_pool = tc.alloc_tile_pool(name="ps2", bufs=2, space="PSUM")
    pools = dict(m1w=m1w_pool, sg=sg_pool, m2w=m2w_pool, osb=osb_pool,
                 ps1=ps1_pool, ps2=ps2_pool)

    gs = gs_pool.tile([P, NSI, TSC], F16)
    xs_sb = xs_pool.tile([P, ND, TSC], F16)

    # ---- Phase S1: gs = silu(sw1 x)*(sw3 x), xs resident, chunk-inner ----
    xsr = xsT[:].rearrange("(dt p) c -> p dt c", p=P)
    nc.sync.dma_start(out=xs_sb[:, :8, 0:TCH], in_=xsr[:, :8, 0:TCH])
    nc.sync.dma_start(out=xs_sb[:, 8:, 0:TCH], in_=xsr[:, 8:, 0:TCH])
    for g in range(math.ceil(NSI / IGRP)):
        w1b, w3b, ng = m1w_group(nc, pools, sw1L, sw3L, g, NSI, True)
        if g == 0:
            nc.sync.dma_start(out=xs_sb[:, :, TCH:TSC], in_=xsr[:, :, TCH:TSC])
            # chunk-major for the first group: both si-tiles work on chunk 0
            # while the chunk-1 DMA is still in flight
            units = [(q, c0) for c0 in range(0, TSC, TCH) for q in range(ng)]
        else:
            units = [(q, c0) for q in range(ng) for c0 in range(0, TSC, TCH)]
        for q, c0 in units:
            mm1_unit(nc, pools, w1b, w3b, q,
                     xs_sb[:, :, c0:c0 + TCH], gs, g * IGRP + q, c0)
    xs_pool.release()

    # ---- Phase R1: ge = silu(w1 xr)*(w3 xr), chunk-outer, xr streamed ----
    ge_pool = tc.alloc_tile_pool(name="ge", bufs=1, side="left")
    ge = ge_pool.tile([P, NI, CAP], F16)
    xrr = xrT[:].rearrange("(dt p) c -> p dt c", p=P)
    for c0 in range(0, CAP, TCH):
        xr_c = xrc_pool.tile([P, ND, TCH], F16, name="xrc", tag="xrc")
        nc.sync.dma_start(out=xr_c[:], in_=xrr[:, :, c0:c0 + TCH])
        for g in range(math.ceil(NI / IGRP)):
            w1b, w3b, ng = m1w_group(nc, pools, w1L, w3L, g, NI, False)
            for q in range(ng):
                mm1_unit(nc, pools, w1b, w3b, q, xr_c[:], ge, g * IGRP + q, c0)

    # ---- Phase S2: z = gs @ sw2 ----
    mm2(nc, pools, gs, sw2L, z_out, NSI, TSC)

    # ---- Phase R2: y = ge @ w2 ----
    mm2(nc, pools, ge, w2L, y_out, NI, CAP)

    ge_pool.release()
    sg_pool.release()
    m1w_pool.release()
    xrc_pool.release()
    gs_pool.release()
    osb_pool.release()
    m2w_pool.release()
    ps2_pool.release()
    ps1_pool.release()


def _declare_io(nc):
    io = {}
    io["xsT"] = nc.dram_tensor("xsT", [D, TSC], F16, kind="ExternalInput").ap()
    io["xrT"] = nc.dram_tensor("xrT", [D, CAP], F16, kind="ExternalInput").ap()
    io["w1L"] = nc.dram_tensor("w1L", [ND, P, I], F16, kind="ExternalInput").ap()
    io["w3L"] = nc.dram_tensor("w3L", [ND, P, I], F16, kind="ExternalInput").ap()
    io["w2L"] = nc.dram_tensor("w2L", [NI, P, D], F16, kind="ExternalInput").ap()
    io["sw1L"] = nc.dram_tensor("sw1L", [ND, P, SI], F16, kind="ExternalInput").ap()
    io["sw3L"] = nc.dram_tensor("sw3L", [ND, P, SI], F16, kind="ExternalInput").ap()
    io["sw2L"] = nc.dram_tensor("sw2L", [NSI, P, D], F16, kind="ExternalInput").ap()
    io["z"] = nc.dram_tensor("z", [TSC, D], F32, kind="ExternalOutput").ap()
    io["y"] = nc.dram_tensor("y", [CAP, D], F32, kind="ExternalOutput").ap()
    return io


@lru_cache(maxsize=1)
def _build():
    nc = bacc.Bacc("TRN2", target_bir_lowering=False, debug=False,
                   num_devices=N_CORES)
    io = _declare_io(nc)
    with tile.TileContext(nc) as tc:
        with ExitStack() as ctx:
            build_moe(nc, tc, ctx, io)
    nc.compile()
    return nc


def host_gate(xt, gate_w):
    """fp32 gate + top-2, matching jax.nn.softmax + lax.top_k semantics."""
    logits = (xt @ gate_w.T.astype(np.float32)).astype(np.float32)
    m = logits.max(axis=1, keepdims=True)
    ex = np.exp(logits - m, dtype=np.float32)
    p = ex / ex.sum(axis=1, keepdims=True, dtype=np.float32)
    # stable argsort of -p == top_k tie-breaking (lower index wins ties)
    order = np.argsort(-p, axis=1, kind="stable")[:, :2]
    return p.astype(np.float32), order


def make_in_maps(x, gate_w, w1, w2, w3, sw1, sw2, sw3):
    xt = np.ascontiguousarray(x.reshape(T, D)).astype(np.float32, copy=False)
    p, order = host_gate(xt, gate_w)

    xT16 = np.ascontiguousarray(xt.T).astype(np.float16)  # [D, T]
    f16 = lambda a: np.ascontiguousarray(a).astype(np.float16)
    shared = dict(
        sw1L=f16(sw1.T).reshape(ND, P, SI),
        sw3L=f16(sw3.T).reshape(ND, P, SI),
        sw2L=f16(sw2.T).reshape(NSI, P, D),
    )
    in_maps = []
    ids_all, w_all, ov_all = [], [], []
    for c in range(N_CORES):
        ids = np.nonzero((order == c).any(axis=1))[0]
        ids_all.append(ids[:CAP])
        w_all.append(p[ids[:CAP], c])
        ov_all.append((ids[CAP:], p[ids[CAP:], c]))
        xrT = np.zeros((D, CAP), np.float16)
        xrT[:, :min(len(ids), CAP)] = xT16[:, ids[:CAP]]
        in_maps.append(dict(
            xsT=np.ascontiguousarray(xT16[:, c * TSC:(c + 1) * TSC]),
            xrT=xrT,
            w1L=f16(w1[c].T).reshape(ND, P, I),
            w3L=f16(w3[c].T).reshape(ND, P, I),
            w2L=f16(w2[c].T).reshape(NI, P, D),
            **shared,
        ))
    return in_maps, ids_all, w_all, ov_all


def _silu(v):
    return v / (1.0 + np.exp(-v))


def combine(res, ids_all, w_all, ov_all, xt, w1, w2, w3, shape):
    out = np.concatenate(
        [res.results[c]["z"] for c in range(N_CORES)], axis=0)  # [T, D] fp32
    for c in range(N_CORES):
        ids, w = ids_all[c], w_all[c]
        out[ids] += w[:, None] * res.results[c]["y"][:len(ids)]
        ov_ids, ov_w = ov_all[c]
        if len(ov_ids):  # overflow rows beyond CAP: exact fp32 on host
            xo = xt[ov_ids]
            h = _silu(xo @ w1[c].T) * (xo @ w3[c].T)
            out[ov_ids] += ov_w[:, None] * (h @ w2[c].T)
    return out.reshape(shape)


def kernel(x, gate_w, w1, w2, w3, sw1, sw2, sw3):
    nc = _build()
    xt = np.ascontiguousarray(x.reshape(T, D)).astype(np.float32, copy=False)
    in_maps, ids_all, w_all, ov_all = make_in_maps(
        x, gate_w, w1, w2, w3, sw1, sw2, sw3)
    res = run_bass_kernel_spmd(nc, in_maps, core_ids=list(range(N_CORES)))
    return combine(res, ids_all, w_all, ov_all, xt,
                   np.asarray(w1, np.float32), np.asarray(w2, np.float32),
                   np.asarray(w3, np.float32), x.shape)
